# revision 18
# baseline (speedup 1.0000x reference)
"""BasicTransformerBlock on 8 TRN2 NeuronCores.

Strategy: pure data parallelism over the batch (B=8, one batch element per
core). Feature-major activations on-chip:
  - all dense projections bf16 x bf16 (weights bf16, activations bf16
    copies; residual stream stays fp32)
  - attention softmax denominator via a ones-column appended to V; the
    [1,N] denominator row is partition-broadcast with a PE ones-matmul
    (no DRAM roundtrip), reciprocal'd, and multiplied into O
  - layernorm stats via ones-vector matmuls; row math on [1,N] tiles;
    rstd and mu*rstd broadcast via PE ones-matmuls
  - FF1 weights streamed bf16 over 3 DMA queues, GEGLU fused, FF2 from
    resident bf16 weights
"""

import math

import numpy as np
import ml_dtypes

import concourse.bass as bass
import concourse.mybir as mybir
import concourse.tile as tile
from concourse import bacc
from concourse.bass_utils import run_bass_kernel_spmd

F32 = mybir.dt.float32
F32R = mybir.dt.float32r
BF16 = mybir.dt.bfloat16
AF = mybir.ActivationFunctionType
OP = mybir.AluOpType

P = 128
B = 8
NT = 1024          # query tokens
D = 640            # model dim; 5 chunks of 128
KC = 5
NH = 8             # heads
DH = 80            # head dim
CM = 77            # context tokens
CD = 768           # context dim; 6 chunks
CKC = 6
FH = 2560          # GEGLU half hidden; 20 chunks of 128
FJ = 20
NC = 2             # token chunks of 512
NW = 512
ISCALE = 1.0 / math.sqrt(DH)
LN_EPS = 1e-5


def _emit(nc, tc, apply_gb):
    d = nc._kd  # dram handles dict
    with (
        tc.tile_pool(name="sb", bufs=1) as sb,
        tc.tile_pool(name="ps", bufs=1, space="PSUM") as ps,
    ):
        _emit_body(nc, tc, d, sb, ps, apply_gb)


def _emit_body(nc, tc, d, sb, ps, apply_gb):
    qs = [nc.sync, nc.scalar, nc.sync, nc.scalar]

    def bank(name):
        return ps.tile([P, NW], F32, tag="bank", bufs=8, name=name)

    # ---------------- critical-path loads: xT + wq + wk per chunk ---------
    xT = sb.tile([P, KC, NT], BF16, tag="resid", bufs=2, name="xT")
    w_sa_q = sb.tile([P, KC, D], BF16, tag="w640", bufs=4, name="w_sa_q")
    w_sa_k = sb.tile([P, KC, D], BF16, tag="w640", bufs=4, name="w_sa_k")
    for c in range(KC):
        qs[c % 2].dma_start(xT[:, c, :],
                            d["xT"].rearrange("(c p) n -> p c n", p=P)[:, c, :])
        qs[(c + 1) % 2].dma_start(
            w_sa_q[:, c, :],
            d["sa_wq"].rearrange("(c p) m -> p c m", p=P)[:, c, :])
        qs[c % 2].dma_start(
            w_sa_k[:, c, :],
            d["sa_wk"].rearrange("(c p) m -> p c m", p=P)[:, c, :])

    # ---------------- constants / biases (gpsimd SWDGE queue) -------------
    # z-broadcast pad tiles: row 0 gets the denominator row, rows 1-127 stay 0
    zpads = [sb.tile([P, NW], BF16, tag="zpad", bufs=2, name=f"zpad{i}")
             for i in range(2)]
    for zp in zpads:
        nc.scalar.dma_start(zp, d["zeros_nw"][:, :])
    e0m = sb.tile([P, P], BF16, tag="e0m", name="e0m")
    nc.scalar.dma_start(e0m, d["e0m"][:, :])
    ones_f = sb.tile([P, P], BF16, tag="onesf", name="ones_f")
    nc.sync.dma_start(ones_f, d["ones_f"][:, :])
    ones_fr = sb.tile([P, P], F32R, tag="onesfr", name="ones_fr")
    nc.sync.dma_start(ones_fr, d["ones_fr"][:, :])
    epst = sb.tile([P, 1], F32, tag="epst", name="epst")
    nc.sync.dma_start(epst, d["epst"][:, :])
    b_sa_bo = sb.tile([P, KC], F32, tag="b1", name="b_sa_bo")
    nc.sync.dma_start(b_sa_bo, d["sa_bo_p"][:, :])
    b_ca_bo = sb.tile([P, KC], F32, tag="b2", name="b_ca_bo")
    nc.sync.dma_start(b_ca_bo, d["ca_bo_p"][:, :])
    b_ff2 = sb.tile([P, KC], F32, tag="b3", name="b_ff2")
    nc.sync.dma_start(b_ff2, d["ff_b2_p"][:, :])
    b_f1a = sb.tile([P, FJ], F32, tag="b4", name="b_f1a")
    nc.sync.dma_start(b_f1a, d["ff_b1a_p"][:, :])
    b_f1g = sb.tile([P, FJ], F32, tag="b5", name="b_f1g")
    nc.sync.dma_start(b_f1g, d["ff_b1g_p"][:, :])
    lngb = {}
    if apply_gb:
        for ln in (1, 2, 3):
            for gb in ("g", "b"):
                t = sb.tile([P, KC], F32, tag=f"ln{ln}{gb}", name=f"ln{ln}{gb}")
                nc.sync.dma_start(t, d[f"ln{ln}_{gb}_p"][:, :])
                lngb[(ln, gb)] = t

    def resid_tile(name):
        return sb.tile([P, KC, NT], BF16, tag="resid", bufs=2, name=name)

    w_sa_v = sb.tile([P, KC, D], BF16, tag="w640", bufs=4, name="w_sa_v")
    for c in range(KC):
        nc.scalar.dma_start(
            w_sa_v[:, c, :],
            d["sa_wv"].rearrange("(c p) m -> p c m", p=P)[:, c, :])

    def qk_proj(h, w_q, w_k, src, pref):
        hs = slice(h * DH, (h + 1) * DH)
        qt = sb.tile([DH, NT], BF16, tag="qk", bufs=6, name=f"qt{pref}_{h}")
        kt = sb.tile([DH, NT], BF16, tag="qk", bufs=6, name=f"kt{pref}_{h}")
        for ncq in range(NC):
            ncs = slice(ncq * NW, (ncq + 1) * NW)
            pq = ps.tile([DH, NW], F32, tag="bank", bufs=8,
                         name=f"pq{pref}_{h}_{ncq}")
            pk = ps.tile([DH, NW], F32, tag="bank", bufs=8,
                         name=f"pk{pref}_{h}_{ncq}")
            for c in range(KC):
                nc.tensor.matmul(pq, w_q[:, c, hs], src[:, c, ncs],
                                 start=(c == 0), stop=(c == KC - 1))
            for c in range(KC):
                nc.tensor.matmul(pk, w_k[:, c, hs], src[:, c, ncs],
                                 start=(c == 0), stop=(c == KC - 1))
            nc.vector.tensor_copy(qt[:, ncs], pq)
            nc.vector.tensor_copy(kt[:, ncs], pk)
        return qt, kt

    # head-0 Q/K first: fills the PE while V weights land
    qk0 = qk_proj(0, w_sa_q, w_sa_k, xT, "s")

    # ---------------- SA: V projection into V_aug -------------------------
    v_aug = sb.tile([P, NH, NH, 97], BF16, tag="vaug", bufs=1, name="v_aug")
    nc.vector.memset(v_aug[:, :, :, 80:96], 0.0)
    nc.vector.memset(v_aug[:, :, :, 96:97], 1.0)
    for tch in range(NH):
        for g in range(2):  # two groups of 4 head-columns (320 wide)
            pv = ps.tile([P, 320], F32, tag="bank", bufs=8, name=f"pv_{tch}_{g}")
            for c in range(KC):
                nc.tensor.matmul(
                    pv, xT[:, c, tch * P:(tch + 1) * P],
                    w_sa_v[:, c, g * 320:(g + 1) * 320],
                    start=(c == 0), stop=(c == KC - 1))
            nc.vector.tensor_copy(
                v_aug[:, tch, 4 * g:4 * g + 4, 0:80],
                pv.rearrange("p (s e) -> p s e", e=80))

    # ---------------- CA: context + K/V weights, projections (hoisted) ----
    ctxT = sb.tile([P, CKC, CM], BF16, tag="ctxT", bufs=1, name="ctxT")
    nc.gpsimd.dma_start(ctxT, d["ctxT_bf"].rearrange("(c p) m -> p c m", p=P))
    w_ca_k = sb.tile([P, CKC, D], BF16, tag="big", bufs=2, name="w_ca_k")
    nc.gpsimd.dma_start(w_ca_k, d["ca_wk"].rearrange("(c p) m -> p c m", p=P))
    w_ca_v = sb.tile([P, CKC, D], BF16, tag="big", bufs=2, name="w_ca_v")
    nc.gpsimd.dma_start(w_ca_v, d["ca_wv"].rearrange("(c p) m -> p c m", p=P))

    kt_ca = sb.tile([DH, NH, CM], BF16, tag="ktca", bufs=1, name="kt_ca")
    for h in range(NH):
        hs = slice(h * DH, (h + 1) * DH)
        pk = ps.tile([DH, CM], F32, tag="bank", bufs=8, name=f"pkca_{h}")
        for c in range(CKC):
            nc.tensor.matmul(pk, w_ca_k[:, c, hs], ctxT[:, c, :],
                             start=(c == 0), stop=(c == CKC - 1))
        nc.vector.tensor_copy(kt_ca[:, h, :], pk)

    vca_aug = sb.tile([CM, NH, 97], BF16, tag="vca", bufs=1, name="vca_aug")
    nc.vector.memset(vca_aug[:, :, 80:96], 0.0)
    nc.vector.memset(vca_aug[:, :, 96:97], 1.0)
    for g in range(2):
        pv = ps.tile([CM, 320], F32, tag="bank", bufs=8, name=f"pvca_{g}")
        for c in range(CKC):
            nc.tensor.matmul(pv, ctxT[:, c, :],
                             w_ca_v[:, c, g * 320:(g + 1) * 320],
                             start=(c == 0), stop=(c == CKC - 1))
        nc.vector.tensor_copy(vca_aug[:, 4 * g:4 * g + 4, 0:80],
                              pv.rearrange("p (s e) -> p s e", e=80))

    w_sa_o = sb.tile([DH, NH, D], BF16, tag="wo", bufs=1, name="w_sa_o")
    nc.gpsimd.dma_start(w_sa_o, d["sa_wo_h"][:, :, :])

    zdram = nc.dram_tensor("zdram", [2 * NH * NC, NW], F32)

    def attn_inner(h, qt, kt, vaug_sl, o_tile, mchunks, mpart, zslot,
                   pe_bcast=False):
        """Interleaved ncq chains: scores -> exp -> AV; z bcast via DRAM
        roundtrip (hidden under later heads) or PE e0-matmul (last head)."""
        ncss = [slice(ncq * NW, (ncq + 1) * NW) for ncq in range(NC)]
        pos = [ps.tile([97, NW], F32, tag="bank", bufs=8,
                       name=f"po_{h}_{ncq}") for ncq in range(NC)]
        ets = [[], []]
        for mc in range(mchunks):
            for ncq in range(NC):
                pscore = ps.tile([mpart, NW], F32, tag="bank", bufs=8,
                                 name=f"psc_{h}_{ncq}_{mc}")
                nc.tensor.matmul(pscore, kt(mc), qt[:, ncss[ncq]],
                                 start=True, stop=True)
                et = sb.tile([mpart, NW], BF16, tag="epool", bufs=4,
                             name=f"e_{h}_{ncq}_{mc}")
                nc.scalar.activation(et, pscore, AF.Exp, scale=ISCALE)
                ets[ncq].append(et)
                if mc >= 1:  # AV one score behind its exp
                    nc.tensor.matmul(pos[ncq], vaug_sl(mc - 1),
                                     ets[ncq][mc - 1],
                                     start=(mc == 1), stop=False,
                                     skip_group_check=True)
        last = mchunks - 1
        for ncq in range(NC):
            po = pos[ncq]
            nc.tensor.matmul(po, vaug_sl(last), ets[ncq][last],
                             start=(mchunks == 1), stop=True,
                             skip_group_check=True)
            zb = sb.tile([DH, NW], F32, tag="zb", bufs=2, name=f"zb_{h}_{ncq}")
            if pe_bcast:
                zp = zpads[ncq % 2]
                nc.vector.tensor_copy(zp[0:1, :], po[96:97, :])
                pzb = ps.tile([DH, NW], F32, tag="bank", bufs=8,
                              name=f"pzb_{h}_{ncq}")
                nc.tensor.matmul(pzb, e0m[:, 0:DH], zp, start=True, stop=True)
                nc.vector.reciprocal_approx_fast(zb, pzb)
            else:
                slot = zslot + ncq
                zrow = sb.tile([1, NW], F32, tag="zrow", bufs=4,
                               name=f"zr_{h}_{ncq}")
                nc.vector.tensor_copy(zrow, po[96:97, :])
                nc.sync.dma_start(zdram[slot:slot + 1, :], zrow)
                nc.sync.dma_start(
                    zb, zdram[slot:slot + 1, :].to_broadcast((DH, NW)))
                nc.vector.reciprocal_approx_fast(zb, zb)
            nc.vector.tensor_tensor(o_tile[:, h, ncss[ncq]], po[0:80, :], zb,
                                    OP.mult)

    # ---------------- SA: per-head QK + attention -------------------------
    o_sa = sb.tile([DH, NH, NT], BF16, tag="opool", bufs=1, name="o_sa")
    for h in range(NH):
        qt, kt = qk0 if h == 0 else qk_proj(h, w_sa_q, w_sa_k, xT, "s")
        attn_inner(
            h, qt,
            kt=lambda mc, _kt=kt: _kt[:, mc * P:(mc + 1) * P],
            vaug_sl=lambda mc, _h=h: v_aug[:, mc, _h, :],
            o_tile=o_sa, mchunks=NH, mpart=P, zslot=h * NC,
            pe_bcast=(h == NH - 1))

    def out_proj(ncq, wo_t, o_tile, bo_t, res_in, res_out, pref):
        ncs = slice(ncq * NW, (ncq + 1) * NW)
        for do in range(KC):
            dos = slice(do * P, (do + 1) * P)
            pr = bank(f"prj_{pref}_{do}_{ncq}")
            for h in range(NH):
                nc.tensor.matmul(pr, wo_t[:, h, dos], o_tile[:, h, ncs],
                                 start=(h == 0), stop=(h == NH - 1))
            nc.vector.scalar_tensor_tensor(
                out=res_out[:, do, ncs], in0=pr, scalar=bo_t[:, do:do + 1],
                in1=res_in[:, do, ncs], op0=OP.add, op1=OP.add)

    # ---------------- layernorm (feature-major, PE-bcast stats) -----------
    def layernorm_nc(rT, ln_idx, ncq):
        ncs = slice(ncq * NW, (ncq + 1) * NW)
        # all-ones lhsT: stats come out replicated on all 128 partitions
        psum_s = ps.tile([P, NW], F32, tag="bank", bufs=8,
                         name=f"ls_{ln_idx}_{ncq}")
        psum_q = ps.tile([P, NW], F32, tag="bank", bufs=8,
                         name=f"lq_{ln_idx}_{ncq}")
        for c in range(KC):
            sq = sb.tile([P, NW], F32R, tag="sq", bufs=2,
                         name=f"sq_{ln_idx}_{ncq}_{c}")
            nc.vector.tensor_tensor(sq, rT[:, c, ncs],
                                    rT[:, c, ncs], OP.mult)
            nc.tensor.matmul(psum_s, ones_f, rT[:, c, ncs],
                             start=(c == 0), stop=(c == KC - 1))
            nc.tensor.matmul(psum_q, ones_fr, sq,
                             start=(c == 0), stop=(c == KC - 1))
        # mu = s/D ; var = q/D - mu^2 ; rstd = 1/sqrt(var+eps)
        mu_b = sb.tile([P, NW], F32, tag="mu_b", bufs=2,
                       name=f"mu_{ln_idx}_{ncq}")
        nc.vector.tensor_scalar_mul(mu_b, psum_s, 1.0 / D)
        t2 = sb.tile([P, NW], F32, tag="t1_b", bufs=4,
                     name=f"t2_{ln_idx}_{ncq}")
        nc.vector.tensor_tensor(t2, mu_b, mu_b, OP.mult)
        wk = sb.tile([P, NW], F32, tag="wk_b", bufs=2,
                     name=f"wk_{ln_idx}_{ncq}")
        nc.vector.scalar_tensor_tensor(
            out=wk, in0=psum_q, scalar=1.0 / D, in1=t2,
            op0=OP.mult, op1=OP.subtract)
        nc.scalar.activation(wk, wk, AF.Sqrt, bias=epst)
        nc.vector.reciprocal_approx_fast(wk, wk)   # wk = rstd
        nc.vector.tensor_tensor(mu_b, mu_b, wk, OP.mult)  # mu_b = mu*rstd
        for c in range(KC):
            t1 = sb.tile([P, NW], F32, tag="t1_b", bufs=4,
                         name=f"t1_{ln_idx}_{ncq}_{c}")
            nc.vector.tensor_tensor(t1, rT[:, c, ncs], wk,
                                    OP.mult)
            if apply_gb:
                t2c = sb.tile([P, NW], F32, tag="t1_b", bufs=4,
                              name=f"t2_{ln_idx}_{ncq}_{c}")
                nc.vector.tensor_tensor(t2c, t1, mu_b, OP.subtract)
                nc.vector.tensor_scalar(
                    out=rT[:, c, ncs], in0=t2c,
                    scalar1=lngb[(ln_idx, "g")][:, c:c + 1],
                    scalar2=lngb[(ln_idx, "b")][:, c:c + 1],
                    op0=OP.mult, op1=OP.add)
            else:
                nc.vector.tensor_tensor(rT[:, c, ncs], t1, mu_b,
                                        OP.subtract)


    # ---------------- SA out-proj + LN1 + bf16 cast, per token chunk ------
    r1T = resid_tile("r1T")
    for ncq in range(NC):
        out_proj(ncq, w_sa_o, o_sa, b_sa_bo, xT, r1T, "r1")
        layernorm_nc(r1T, 1, ncq)
    x1T = r1T

    # ---------------- CA: Q weights + out weights -------------------------
    w_ca_q = sb.tile([P, KC, D], BF16, tag="w640", bufs=4, name="w_ca_q")
    nc.gpsimd.dma_start(w_ca_q, d["ca_wq"].rearrange("(c p) m -> p c m", p=P))
    w_ca_o = sb.tile([DH, NH, D], BF16, tag="wo", bufs=1, name="w_ca_o")
    nc.gpsimd.dma_start(w_ca_o, d["ca_wo_h"][:, :, :])

    # ---------------- CA: per-head Q + attention --------------------------
    o_ca = sb.tile([DH, NH, NT], BF16, tag="opool", bufs=1, name="o_ca")
    qtcas = []
    for h in range(NH):
        hs = slice(h * DH, (h + 1) * DH)
        qt = sb.tile([DH, NT], BF16, tag="qk", bufs=6, name=f"qtca_{h}")
        for ncq in range(NC):
            ncs = slice(ncq * NW, (ncq + 1) * NW)
            pq = ps.tile([DH, NW], F32, tag="bank", bufs=8,
                         name=f"pqca_{h}_{ncq}")
            for c in range(KC):
                nc.tensor.matmul(pq, w_ca_q[:, c, hs], x1T[:, c, ncs],
                                 start=(c == 0), stop=(c == KC - 1))
            nc.vector.tensor_copy(qt[:, ncs], pq)
        qtcas.append(qt)
    for h in range(NH):
        attn_inner(
            h, qtcas[h],
            kt=lambda mc, _h=h: kt_ca[:, _h, :],
            vaug_sl=lambda mc, _h=h: vca_aug[:, _h, :],
            o_tile=o_ca, mchunks=1, mpart=CM, zslot=NH * NC + h * NC,
            pe_bcast=(h == NH - 1))

    # ---------------- FF2 weights early (during CA) -----------------------
    w_ff2 = []
    for t in range(4):
        wt = sb.tile([P, KC, D], BF16, tag="w640", bufs=4, name=f"w_ff2_{t}")
        nc.gpsimd.dma_start(
            wt, d["ff_w2"].rearrange("(t c p) m -> t p c m", p=P, c=KC)[t])
        w_ff2.append(wt)

    # ---------------- FF1 weight prefetch (first 4 j) ---------------------
    ffw = {}

    def ff_dma(j):
        wja = sb.tile([P, KC, P], BF16, tag="wff1", bufs=8, name=f"wja_{j}")
        nc.scalar.dma_start(
            wja, d["ff_w1"].rearrange("(c p) m -> p c m", p=P)
            [:, :, j * P:(j + 1) * P])
        wjg = sb.tile([P, KC, P], BF16, tag="wff1", bufs=8, name=f"wjg_{j}")
        nc.gpsimd.dma_start(
            wjg, d["ff_w1"].rearrange("(c p) m -> p c m", p=P)
            [:, :, FH + j * P:FH + (j + 1) * P])
        ffw[j] = (wja, wjg)

    for j in range(4):
        ff_dma(j)

    # ---------------- CA out-proj + LN2 -----------------------------------
    r2T = resid_tile("r2T")
    for ncq in range(NC):
        out_proj(ncq, w_ca_o, o_ca, b_ca_bo, x1T, r2T, "r2")
        layernorm_nc(r2T, 2, ncq)
    x2T = r2T

    # ---------------- FF (GEGLU): both token chunks per j -----------------
    mfulls = [
        sb.tile([P, FJ, NW], BF16, tag="big", bufs=2, name=f"mfull_{ncq}")
        for ncq in range(NC)
    ]
    for j in range(FJ):
        if j not in ffw:
            ff_dma(j)
        wja, wjg = ffw[j]
        for ncq in range(NC):
            ncs = slice(ncq * NW, (ncq + 1) * NW)
            pa = bank(f"pa_{ncq}_{j}")
            pg = bank(f"pg_{ncq}_{j}")
            for c in range(KC):
                nc.tensor.matmul(pa, wja[:, c, :], x2T[:, c, ncs],
                                 start=(c == 0), stop=(c == KC - 1))
            for c in range(KC):
                nc.tensor.matmul(pg, wjg[:, c, :], x2T[:, c, ncs],
                                 start=(c == 0), stop=(c == KC - 1))
            gj = sb.tile([P, NW], BF16, tag="gelu", bufs=2, name=f"gj_{ncq}_{j}")
            nc.scalar.activation(gj, pg, AF.Gelu, bias=b_f1g[:, j:j + 1])
            nc.vector.scalar_tensor_tensor(
                out=mfulls[ncq][:, j, :], in0=pa, scalar=b_f1a[:, j:j + 1],
                in1=gj, op0=OP.add, op1=OP.mult)

    # ---------------- FF2 + LN3 + store -----------------------------------
    r3T = resid_tile("r3T")
    for ncq in range(NC):
        ncs = slice(ncq * NW, (ncq + 1) * NW)
        for do in range(KC):
            dos = slice(do * P, (do + 1) * P)
            pr = bank(f"pr3_{do}_{ncq}")
            for j in range(FJ):
                nc.tensor.matmul(pr, w_ff2[j // KC][:, j % KC, dos],
                                 mfulls[ncq][:, j, :],
                                 start=(j == 0), stop=(j == FJ - 1))
            nc.vector.scalar_tensor_tensor(
                out=r3T[:, do, ncs], in0=pr, scalar=b_ff2[:, do:do + 1],
                in1=x2T[:, do, ncs], op0=OP.add, op1=OP.add)
        layernorm_nc(r3T, 3, ncq)
        for c in range(KC):
            nc.sync.dma_start(
                d["outT"].rearrange("(c p) n -> p c n", p=P)[:, c, ncs],
                r3T[:, c, ncs])


def _build(apply_gb):
    nc = bacc.Bacc(None, target_bir_lowering=False)
    dt_in = [
        ("xT", [D, NT], BF16),
        ("ctxT_bf", [CD, CM], BF16),
        ("sa_wq", [D, D], BF16), ("sa_wk", [D, D], BF16),
        ("sa_wv", [D, D], BF16), ("sa_wo_h", [DH, NH, D], BF16),
        ("ca_wq", [D, D], BF16), ("ca_wk", [CD, D], BF16),
        ("ca_wv", [CD, D], BF16), ("ca_wo_h", [DH, NH, D], BF16),
        ("ff_w1", [D, 2 * FH], BF16), ("ff_w2", [FH, D], BF16),
        ("sa_bo_p", [P, KC], F32), ("ca_bo_p", [P, KC], F32),
        ("ff_b2_p", [P, KC], F32),
        ("ff_b1a_p", [P, FJ], F32), ("ff_b1g_p", [P, FJ], F32),
        ("ones_f", [P, P], BF16), ("ones_fr", [P, P], F32R),
        ("e0m", [P, P], BF16),
        ("zeros_nw", [P, NW], BF16), ("epst", [P, 1], F32),
    ]
    if apply_gb:
        for ln in (1, 2, 3):
            dt_in.append((f"ln{ln}_g_p", [P, KC], F32))
            dt_in.append((f"ln{ln}_b_p", [P, KC], F32))
    nc._kd = {}
    for name, shape, dt in dt_in:
        nc._kd[name] = nc.declare_dram_parameter(name, shape, dt,
                                                 isOutput=False)
    nc._kd["outT"] = nc.declare_dram_parameter("outT", [D, NT], BF16,
                                               isOutput=True)
    with tile.TileContext(nc) as tc:
        _emit(nc, tc, apply_gb)
    nc.compile()
    return nc


def _prep_in_maps(inputs, apply_gb):
    f32 = np.float32
    bf = ml_dtypes.bfloat16
    x = np.asarray(inputs["x"], f32)
    ctx = np.asarray(inputs["context"], f32)

    def heads(w):
        # [640, 640] -> [80, 8, 640] head-major partition layout
        return np.ascontiguousarray(
            np.asarray(w, f32).reshape(NH, DH, D).transpose(1, 0, 2)
        ).astype(bf)

    def part(v, cols):
        return np.ascontiguousarray(np.asarray(v, f32).reshape(cols, P).T)

    shared = {
        "sa_wq": np.asarray(inputs["sa_wq"], f32).astype(bf),
        "sa_wk": np.asarray(inputs["sa_wk"], f32).astype(bf),
        "sa_wv": np.asarray(inputs["sa_wv"], f32).astype(bf),
        "sa_wo_h": heads(inputs["sa_wo"]),
        "ca_wq": np.asarray(inputs["ca_wq"], f32).astype(bf),
        "ca_wk": np.asarray(inputs["ca_wk"], f32).astype(bf),
        "ca_wv": np.asarray(inputs["ca_wv"], f32).astype(bf),
        "ca_wo_h": heads(inputs["ca_wo"]),
        "ff_w1": np.asarray(inputs["ff_w1"], f32).astype(bf),
        "ff_w2": np.asarray(inputs["ff_w2"], f32).astype(bf),
        "sa_bo_p": part(inputs["sa_bo"], KC),
        "ca_bo_p": part(inputs["ca_bo"], KC),
        "ff_b2_p": part(inputs["ff_b2"], KC),
        "ff_b1a_p": part(np.asarray(inputs["ff_b1"], f32)[:FH], FJ),
        "ff_b1g_p": part(np.asarray(inputs["ff_b1"], f32)[FH:], FJ),
        "ones_f": np.ones((P, P), bf),
        "ones_fr": np.ones((P, P), f32),
        "e0m": np.concatenate([np.ones((1, P), bf),
                               np.zeros((P - 1, P), bf)], axis=0),
        "zeros_nw": np.zeros((P, NW), bf),
        "epst": np.full((P, 1), LN_EPS, f32),
    }
    if apply_gb:
        for ln in (1, 2, 3):
            shared[f"ln{ln}_g_p"] = part(inputs[f"ln{ln}_g"], KC)
            shared[f"ln{ln}_b_p"] = part(inputs[f"ln{ln}_b"], KC)
    maps = []
    for i in range(B):
        m = dict(shared)
        m["xT"] = np.ascontiguousarray(x[i].T).astype(bf)
        m["ctxT_bf"] = np.ascontiguousarray(ctx[i].T).astype(bf)
        maps.append(m)
    return maps


def _needs_gb(inputs):
    for ln in (1, 2, 3):
        if not np.allclose(np.asarray(inputs[f"ln{ln}_g"]), 1.0):
            return True
        if not np.allclose(np.asarray(inputs[f"ln{ln}_b"]), 0.0):
            return True
    return False


def _run(inputs, trace=False):
    apply_gb = _needs_gb(inputs)
    nc = _build(apply_gb)
    maps = _prep_in_maps(inputs, apply_gb)
    res = run_bass_kernel_spmd(nc, maps, core_ids=list(range(B)), trace=trace)
    out = np.stack([np.asarray(r["outT"]).T for r in res.results])
    return out.astype(np.float32), res


def kernel(**inputs):
    out, _ = _run(inputs, trace=False)
    return out


# revision 19
# speedup vs baseline: 1.0141x; 1.0141x over previous
"""BasicTransformerBlock on 8 TRN2 NeuronCores.

Strategy: pure data parallelism over the batch (B=8, one batch element per
core). Feature-major activations on-chip:
  - all dense projections bf16 x bf16 (weights bf16, activations bf16
    copies; residual stream stays fp32)
  - attention softmax denominator via a ones-column appended to V; the
    [1,N] denominator row is partition-broadcast with a PE ones-matmul
    (no DRAM roundtrip), reciprocal'd, and multiplied into O
  - layernorm stats via ones-vector matmuls; row math on [1,N] tiles;
    rstd and mu*rstd broadcast via PE ones-matmuls
  - FF1 weights streamed bf16 over 3 DMA queues, GEGLU fused, FF2 from
    resident bf16 weights
"""

import math

import numpy as np
import ml_dtypes

import concourse.bass as bass
import concourse.mybir as mybir
import concourse.tile as tile
from concourse import bacc
from concourse.bass_utils import run_bass_kernel_spmd

F32 = mybir.dt.float32
F32R = mybir.dt.float32r
BF16 = mybir.dt.bfloat16
AF = mybir.ActivationFunctionType
OP = mybir.AluOpType

P = 128
B = 8
NT = 1024          # query tokens
D = 640            # model dim; 5 chunks of 128
KC = 5
NH = 8             # heads
DH = 80            # head dim
CM = 77            # context tokens
CD = 768           # context dim; 6 chunks
CKC = 6
FH = 2560          # GEGLU half hidden; 20 chunks of 128
FJ = 20
NC = 2             # token chunks of 512
NW = 512
ISCALE = 1.0 / math.sqrt(DH)
LN_EPS = 1e-5


def _emit(nc, tc, apply_gb):
    d = nc._kd  # dram handles dict
    with (
        tc.tile_pool(name="sb", bufs=1) as sb,
        tc.tile_pool(name="ps", bufs=1, space="PSUM") as ps,
    ):
        _emit_body(nc, tc, d, sb, ps, apply_gb)


def _emit_body(nc, tc, d, sb, ps, apply_gb):
    qs = [nc.sync, nc.scalar, nc.sync, nc.scalar]

    def bank(name):
        return ps.tile([P, NW], F32, tag="bank", bufs=8, name=name)

    # ---------------- critical-path loads: xT + wq + wk per chunk ---------
    xT = sb.tile([P, KC, NT], BF16, tag="resid", bufs=2, name="xT")
    w_sa_q = sb.tile([P, KC, D], BF16, tag="w640", bufs=4, name="w_sa_q")
    w_sa_k = sb.tile([P, KC, D], BF16, tag="w640", bufs=4, name="w_sa_k")
    for c in range(KC):
        qs[c % 2].dma_start(xT[:, c, :],
                            d["xT"].rearrange("(c p) n -> p c n", p=P)[:, c, :])
        nc.gpsimd.dma_start(
            w_sa_q[:, c, :],
            d["sa_wq"].rearrange("(c p) m -> p c m", p=P)[:, c, :])
        qs[(c + 1) % 2].dma_start(
            w_sa_k[:, c, :],
            d["sa_wk"].rearrange("(c p) m -> p c m", p=P)[:, c, :])

    # ---------------- constants / biases (gpsimd SWDGE queue) -------------
    # z-broadcast pad tiles: row 0 gets the denominator row, rows 1-127 stay 0
    zpads = [sb.tile([P, NW], BF16, tag="zpad", bufs=2, name=f"zpad{i}")
             for i in range(2)]
    for zp in zpads:
        nc.scalar.dma_start(zp, d["zeros_nw"][:, :])
    e0m = sb.tile([P, P], BF16, tag="e0m", name="e0m")
    nc.scalar.dma_start(e0m, d["e0m"][:, :])
    ones_f = sb.tile([P, P], BF16, tag="onesf", name="ones_f")
    nc.sync.dma_start(ones_f, d["ones_f"][:, :])
    ones_fr = sb.tile([P, P], F32R, tag="onesfr", name="ones_fr")
    nc.sync.dma_start(ones_fr, d["ones_fr"][:, :])
    epst = sb.tile([P, 1], F32, tag="epst", name="epst")
    nc.sync.dma_start(epst, d["epst"][:, :])
    b_sa_bo = sb.tile([P, KC], F32, tag="b1", name="b_sa_bo")
    nc.sync.dma_start(b_sa_bo, d["sa_bo_p"][:, :])
    b_ca_bo = sb.tile([P, KC], F32, tag="b2", name="b_ca_bo")
    nc.sync.dma_start(b_ca_bo, d["ca_bo_p"][:, :])
    b_ff2 = sb.tile([P, KC], F32, tag="b3", name="b_ff2")
    nc.sync.dma_start(b_ff2, d["ff_b2_p"][:, :])
    b_f1a = sb.tile([P, FJ], F32, tag="b4", name="b_f1a")
    nc.sync.dma_start(b_f1a, d["ff_b1a_p"][:, :])
    b_f1g = sb.tile([P, FJ], F32, tag="b5", name="b_f1g")
    nc.sync.dma_start(b_f1g, d["ff_b1g_p"][:, :])
    lngb = {}
    if apply_gb:
        for ln in (1, 2, 3):
            for gb in ("g", "b"):
                t = sb.tile([P, KC], F32, tag=f"ln{ln}{gb}", name=f"ln{ln}{gb}")
                nc.sync.dma_start(t, d[f"ln{ln}_{gb}_p"][:, :])
                lngb[(ln, gb)] = t

    def resid_tile(name):
        return sb.tile([P, KC, NT], BF16, tag="resid", bufs=2, name=name)

    w_sa_v = sb.tile([P, KC, D], BF16, tag="w640", bufs=4, name="w_sa_v")
    for c in range(KC):
        nc.gpsimd.dma_start(
            w_sa_v[:, c, :],
            d["sa_wv"].rearrange("(c p) m -> p c m", p=P)[:, c, :])

    def qk_proj(h, w_q, w_k, src, pref):
        hs = slice(h * DH, (h + 1) * DH)
        qt = sb.tile([DH, NT], BF16, tag="qk", bufs=6, name=f"qt{pref}_{h}")
        kt = sb.tile([DH, NT], BF16, tag="qk", bufs=6, name=f"kt{pref}_{h}")
        for ncq in range(NC):
            ncs = slice(ncq * NW, (ncq + 1) * NW)
            pq = ps.tile([DH, NW], F32, tag="bank", bufs=8,
                         name=f"pq{pref}_{h}_{ncq}")
            pk = ps.tile([DH, NW], F32, tag="bank", bufs=8,
                         name=f"pk{pref}_{h}_{ncq}")
            for c in range(KC):
                nc.tensor.matmul(pq, w_q[:, c, hs], src[:, c, ncs],
                                 start=(c == 0), stop=(c == KC - 1))
            for c in range(KC):
                nc.tensor.matmul(pk, w_k[:, c, hs], src[:, c, ncs],
                                 start=(c == 0), stop=(c == KC - 1))
            nc.vector.tensor_copy(qt[:, ncs], pq)
            nc.vector.tensor_copy(kt[:, ncs], pk)
        return qt, kt

    # head-0 Q/K first: fills the PE while V weights land
    qk0 = qk_proj(0, w_sa_q, w_sa_k, xT, "s")

    # ---------------- SA: V projection into V_aug -------------------------
    v_aug = sb.tile([P, NH, NH, 97], BF16, tag="vaug", bufs=1, name="v_aug")
    nc.vector.memset(v_aug[:, :, :, 80:96], 0.0)
    nc.vector.memset(v_aug[:, :, :, 96:97], 1.0)
    for tch in range(NH):
        for g in range(2):  # two groups of 4 head-columns (320 wide)
            pv = ps.tile([P, 320], F32, tag="bank", bufs=8, name=f"pv_{tch}_{g}")
            for c in range(KC):
                nc.tensor.matmul(
                    pv, xT[:, c, tch * P:(tch + 1) * P],
                    w_sa_v[:, c, g * 320:(g + 1) * 320],
                    start=(c == 0), stop=(c == KC - 1))
            nc.vector.tensor_copy(
                v_aug[:, tch, 4 * g:4 * g + 4, 0:80],
                pv.rearrange("p (s e) -> p s e", e=80))

    # ---------------- CA: context + K/V weights, projections (hoisted) ----
    ctxT = sb.tile([P, CKC, CM], BF16, tag="ctxT", bufs=1, name="ctxT")
    nc.scalar.dma_start(ctxT, d["ctxT_bf"].rearrange("(c p) m -> p c m", p=P))
    w_ca_k = sb.tile([P, CKC, D], BF16, tag="big", bufs=2, name="w_ca_k")
    nc.scalar.dma_start(w_ca_k, d["ca_wk"].rearrange("(c p) m -> p c m", p=P))
    w_ca_v = sb.tile([P, CKC, D], BF16, tag="big", bufs=2, name="w_ca_v")
    nc.scalar.dma_start(w_ca_v, d["ca_wv"].rearrange("(c p) m -> p c m", p=P))

    kt_ca = sb.tile([DH, NH, CM], BF16, tag="ktca", bufs=1, name="kt_ca")
    for h in range(NH):
        hs = slice(h * DH, (h + 1) * DH)
        pk = ps.tile([DH, CM], F32, tag="bank", bufs=8, name=f"pkca_{h}")
        for c in range(CKC):
            nc.tensor.matmul(pk, w_ca_k[:, c, hs], ctxT[:, c, :],
                             start=(c == 0), stop=(c == CKC - 1))
        nc.vector.tensor_copy(kt_ca[:, h, :], pk)

    vca_aug = sb.tile([CM, NH, 97], BF16, tag="vca", bufs=1, name="vca_aug")
    nc.vector.memset(vca_aug[:, :, 80:96], 0.0)
    nc.vector.memset(vca_aug[:, :, 96:97], 1.0)
    for g in range(2):
        pv = ps.tile([CM, 320], F32, tag="bank", bufs=8, name=f"pvca_{g}")
        for c in range(CKC):
            nc.tensor.matmul(pv, ctxT[:, c, :],
                             w_ca_v[:, c, g * 320:(g + 1) * 320],
                             start=(c == 0), stop=(c == CKC - 1))
        nc.vector.tensor_copy(vca_aug[:, 4 * g:4 * g + 4, 0:80],
                              pv.rearrange("p (s e) -> p s e", e=80))

    w_sa_o = sb.tile([DH, NH, D], BF16, tag="wo", bufs=1, name="w_sa_o")
    nc.gpsimd.dma_start(w_sa_o, d["sa_wo_h"][:, :, :])

    zdram = nc.dram_tensor("zdram", [2 * NH * NC, NW], F32)

    def attn_inner(h, qt, kt, vaug_sl, o_tile, mchunks, mpart, zslot,
                   pe_bcast=False):
        """Interleaved ncq chains: scores -> exp -> AV; z bcast via DRAM
        roundtrip (hidden under later heads) or PE e0-matmul (last head)."""
        ncss = [slice(ncq * NW, (ncq + 1) * NW) for ncq in range(NC)]
        pos = [ps.tile([97, NW], F32, tag="bank", bufs=8,
                       name=f"po_{h}_{ncq}") for ncq in range(NC)]
        ets = [[], []]
        for mc in range(mchunks):
            for ncq in range(NC):
                pscore = ps.tile([mpart, NW], F32, tag="bank", bufs=8,
                                 name=f"psc_{h}_{ncq}_{mc}")
                nc.tensor.matmul(pscore, kt(mc), qt[:, ncss[ncq]],
                                 start=True, stop=True)
                et = sb.tile([mpart, NW], BF16, tag="epool", bufs=4,
                             name=f"e_{h}_{ncq}_{mc}")
                nc.scalar.activation(et, pscore, AF.Exp, scale=ISCALE)
                ets[ncq].append(et)
                if mc >= 1:  # AV one score behind its exp
                    nc.tensor.matmul(pos[ncq], vaug_sl(mc - 1),
                                     ets[ncq][mc - 1],
                                     start=(mc == 1), stop=False,
                                     skip_group_check=True)
        last = mchunks - 1
        for ncq in range(NC):
            po = pos[ncq]
            nc.tensor.matmul(po, vaug_sl(last), ets[ncq][last],
                             start=(mchunks == 1), stop=True,
                             skip_group_check=True)
            zb = sb.tile([DH, NW], F32, tag="zb", bufs=2, name=f"zb_{h}_{ncq}")
            if pe_bcast:
                zp = zpads[ncq % 2]
                nc.vector.tensor_copy(zp[0:1, :], po[96:97, :])
                pzb = ps.tile([DH, NW], F32, tag="bank", bufs=8,
                              name=f"pzb_{h}_{ncq}")
                nc.tensor.matmul(pzb, e0m[:, 0:DH], zp, start=True, stop=True)
                nc.vector.reciprocal_approx_fast(zb, pzb)
            else:
                slot = zslot + ncq
                zrow = sb.tile([1, NW], F32, tag="zrow", bufs=4,
                               name=f"zr_{h}_{ncq}")
                nc.vector.tensor_copy(zrow, po[96:97, :])
                nc.sync.dma_start(zdram[slot:slot + 1, :], zrow)
                nc.sync.dma_start(
                    zb, zdram[slot:slot + 1, :].to_broadcast((DH, NW)))
                nc.vector.reciprocal_approx_fast(zb, zb)
            nc.vector.tensor_tensor(o_tile[:, h, ncss[ncq]], po[0:80, :], zb,
                                    OP.mult)

    # ---------------- SA: per-head QK + attention -------------------------
    o_sa = sb.tile([DH, NH, NT], BF16, tag="opool", bufs=1, name="o_sa")
    for h in range(NH):
        qt, kt = qk0 if h == 0 else qk_proj(h, w_sa_q, w_sa_k, xT, "s")
        attn_inner(
            h, qt,
            kt=lambda mc, _kt=kt: _kt[:, mc * P:(mc + 1) * P],
            vaug_sl=lambda mc, _h=h: v_aug[:, mc, _h, :],
            o_tile=o_sa, mchunks=NH, mpart=P, zslot=h * NC,
            pe_bcast=(h == NH - 1))

    def out_proj(ncq, wo_t, o_tile, bo_t, res_in, res_out, pref):
        ncs = slice(ncq * NW, (ncq + 1) * NW)
        for do in range(KC):
            dos = slice(do * P, (do + 1) * P)
            pr = bank(f"prj_{pref}_{do}_{ncq}")
            for h in range(NH):
                nc.tensor.matmul(pr, wo_t[:, h, dos], o_tile[:, h, ncs],
                                 start=(h == 0), stop=(h == NH - 1))
            nc.vector.scalar_tensor_tensor(
                out=res_out[:, do, ncs], in0=pr, scalar=bo_t[:, do:do + 1],
                in1=res_in[:, do, ncs], op0=OP.add, op1=OP.add)

    # ---------------- layernorm (feature-major, PE-bcast stats) -----------
    def layernorm_nc(rT, ln_idx, ncq):
        ncs = slice(ncq * NW, (ncq + 1) * NW)
        # all-ones lhsT: stats come out replicated on all 128 partitions
        psum_s = ps.tile([P, NW], F32, tag="bank", bufs=8,
                         name=f"ls_{ln_idx}_{ncq}")
        psum_q = ps.tile([P, NW], F32, tag="bank", bufs=8,
                         name=f"lq_{ln_idx}_{ncq}")
        for c in range(KC):
            sq = sb.tile([P, NW], F32R, tag="sq", bufs=2,
                         name=f"sq_{ln_idx}_{ncq}_{c}")
            nc.vector.tensor_tensor(sq, rT[:, c, ncs],
                                    rT[:, c, ncs], OP.mult)
            nc.tensor.matmul(psum_s, ones_f, rT[:, c, ncs],
                             start=(c == 0), stop=(c == KC - 1))
            nc.tensor.matmul(psum_q, ones_fr, sq,
                             start=(c == 0), stop=(c == KC - 1))
        # mu = s/D ; var = q/D - mu^2 ; rstd = 1/sqrt(var+eps)
        mu_b = sb.tile([P, NW], F32, tag="mu_b", bufs=2,
                       name=f"mu_{ln_idx}_{ncq}")
        nc.vector.tensor_scalar_mul(mu_b, psum_s, 1.0 / D)
        t2 = sb.tile([P, NW], F32, tag="t1_b", bufs=4,
                     name=f"t2_{ln_idx}_{ncq}")
        nc.vector.tensor_tensor(t2, mu_b, mu_b, OP.mult)
        wk = sb.tile([P, NW], F32, tag="wk_b", bufs=2,
                     name=f"wk_{ln_idx}_{ncq}")
        nc.vector.scalar_tensor_tensor(
            out=wk, in0=psum_q, scalar=1.0 / D, in1=t2,
            op0=OP.mult, op1=OP.subtract)
        nc.scalar.activation(wk, wk, AF.Sqrt, bias=epst)
        nc.vector.reciprocal_approx_fast(wk, wk)   # wk = rstd
        nc.vector.tensor_tensor(mu_b, mu_b, wk, OP.mult)  # mu_b = mu*rstd
        for c in range(KC):
            t1 = sb.tile([P, NW], F32, tag="t1_b", bufs=4,
                         name=f"t1_{ln_idx}_{ncq}_{c}")
            nc.vector.tensor_tensor(t1, rT[:, c, ncs], wk,
                                    OP.mult)
            if apply_gb:
                t2c = sb.tile([P, NW], F32, tag="t1_b", bufs=4,
                              name=f"t2_{ln_idx}_{ncq}_{c}")
                nc.vector.tensor_tensor(t2c, t1, mu_b, OP.subtract)
                nc.vector.tensor_scalar(
                    out=rT[:, c, ncs], in0=t2c,
                    scalar1=lngb[(ln_idx, "g")][:, c:c + 1],
                    scalar2=lngb[(ln_idx, "b")][:, c:c + 1],
                    op0=OP.mult, op1=OP.add)
            else:
                nc.vector.tensor_tensor(rT[:, c, ncs], t1, mu_b,
                                        OP.subtract)


    # ---------------- SA out-proj + LN1 + bf16 cast, per token chunk ------
    r1T = resid_tile("r1T")
    for ncq in range(NC):
        out_proj(ncq, w_sa_o, o_sa, b_sa_bo, xT, r1T, "r1")
        layernorm_nc(r1T, 1, ncq)
    x1T = r1T

    # ---------------- CA: Q weights + out weights -------------------------
    w_ca_q = sb.tile([P, KC, D], BF16, tag="w640", bufs=4, name="w_ca_q")
    nc.gpsimd.dma_start(w_ca_q, d["ca_wq"].rearrange("(c p) m -> p c m", p=P))
    w_ca_o = sb.tile([DH, NH, D], BF16, tag="wo", bufs=1, name="w_ca_o")
    nc.gpsimd.dma_start(w_ca_o, d["ca_wo_h"][:, :, :])

    # ---------------- CA: per-head Q + attention --------------------------
    o_ca = sb.tile([DH, NH, NT], BF16, tag="opool", bufs=1, name="o_ca")
    for h in range(NH):
        hs = slice(h * DH, (h + 1) * DH)
        qt = sb.tile([DH, NT], BF16, tag="qk", bufs=6, name=f"qtca_{h}")
        for ncq in range(NC):
            ncs = slice(ncq * NW, (ncq + 1) * NW)
            pq = ps.tile([DH, NW], F32, tag="bank", bufs=8,
                         name=f"pqca_{h}_{ncq}")
            for c in range(KC):
                nc.tensor.matmul(pq, w_ca_q[:, c, hs], x1T[:, c, ncs],
                                 start=(c == 0), stop=(c == KC - 1))
            nc.vector.tensor_copy(qt[:, ncs], pq)
        attn_inner(
            h, qt,
            kt=lambda mc, _h=h: kt_ca[:, _h, :],
            vaug_sl=lambda mc, _h=h: vca_aug[:, _h, :],
            o_tile=o_ca, mchunks=1, mpart=CM, zslot=NH * NC + h * NC,
            pe_bcast=True)

    # ---------------- FF2 weights early (during CA) -----------------------
    w_ff2 = []
    for t in range(4):
        wt = sb.tile([P, KC, D], BF16, tag="w640", bufs=4, name=f"w_ff2_{t}")
        nc.gpsimd.dma_start(
            wt, d["ff_w2"].rearrange("(t c p) m -> t p c m", p=P, c=KC)[t])
        w_ff2.append(wt)

    # ---------------- FF1 weight prefetch (first 4 j) ---------------------
    ffw = {}

    def ff_dma(j):
        wja = sb.tile([P, KC, P], BF16, tag="wff1", bufs=8, name=f"wja_{j}")
        nc.scalar.dma_start(
            wja, d["ff_w1"].rearrange("(c p) m -> p c m", p=P)
            [:, :, j * P:(j + 1) * P])
        wjg = sb.tile([P, KC, P], BF16, tag="wff1", bufs=8, name=f"wjg_{j}")
        nc.gpsimd.dma_start(
            wjg, d["ff_w1"].rearrange("(c p) m -> p c m", p=P)
            [:, :, FH + j * P:FH + (j + 1) * P])
        ffw[j] = (wja, wjg)

    for j in range(4):
        ff_dma(j)

    # ---------------- CA out-proj + LN2 -----------------------------------
    r2T = resid_tile("r2T")
    for ncq in range(NC):
        out_proj(ncq, w_ca_o, o_ca, b_ca_bo, x1T, r2T, "r2")
        layernorm_nc(r2T, 2, ncq)
    x2T = r2T

    # ---------------- FF (GEGLU): both token chunks per j -----------------
    mfulls = [
        sb.tile([P, FJ, NW], BF16, tag="big", bufs=2, name=f"mfull_{ncq}")
        for ncq in range(NC)
    ]
    for j in range(FJ):
        if j not in ffw:
            ff_dma(j)
        wja, wjg = ffw[j]
        for ncq in range(NC):
            ncs = slice(ncq * NW, (ncq + 1) * NW)
            pa = bank(f"pa_{ncq}_{j}")
            pg = bank(f"pg_{ncq}_{j}")
            for c in range(KC):
                nc.tensor.matmul(pa, wja[:, c, :], x2T[:, c, ncs],
                                 start=(c == 0), stop=(c == KC - 1))
            for c in range(KC):
                nc.tensor.matmul(pg, wjg[:, c, :], x2T[:, c, ncs],
                                 start=(c == 0), stop=(c == KC - 1))
            gj = sb.tile([P, NW], BF16, tag="gelu", bufs=2, name=f"gj_{ncq}_{j}")
            nc.scalar.activation(gj, pg, AF.Gelu, bias=b_f1g[:, j:j + 1])
            nc.vector.scalar_tensor_tensor(
                out=mfulls[ncq][:, j, :], in0=pa, scalar=b_f1a[:, j:j + 1],
                in1=gj, op0=OP.add, op1=OP.mult)

    # ---------------- FF2 + LN3 + store -----------------------------------
    r3T = resid_tile("r3T")
    for ncq in range(NC):
        ncs = slice(ncq * NW, (ncq + 1) * NW)
        for do in range(KC):
            dos = slice(do * P, (do + 1) * P)
            pr = bank(f"pr3_{do}_{ncq}")
            for j in range(FJ):
                nc.tensor.matmul(pr, w_ff2[j // KC][:, j % KC, dos],
                                 mfulls[ncq][:, j, :],
                                 start=(j == 0), stop=(j == FJ - 1))
            nc.vector.scalar_tensor_tensor(
                out=r3T[:, do, ncs], in0=pr, scalar=b_ff2[:, do:do + 1],
                in1=x2T[:, do, ncs], op0=OP.add, op1=OP.add)
        layernorm_nc(r3T, 3, ncq)
        for c in range(KC):
            nc.sync.dma_start(
                d["outT"].rearrange("(c p) n -> p c n", p=P)[:, c, ncs],
                r3T[:, c, ncs])


def _build(apply_gb):
    nc = bacc.Bacc(None, target_bir_lowering=False)
    dt_in = [
        ("xT", [D, NT], BF16),
        ("ctxT_bf", [CD, CM], BF16),
        ("sa_wq", [D, D], BF16), ("sa_wk", [D, D], BF16),
        ("sa_wv", [D, D], BF16), ("sa_wo_h", [DH, NH, D], BF16),
        ("ca_wq", [D, D], BF16), ("ca_wk", [CD, D], BF16),
        ("ca_wv", [CD, D], BF16), ("ca_wo_h", [DH, NH, D], BF16),
        ("ff_w1", [D, 2 * FH], BF16), ("ff_w2", [FH, D], BF16),
        ("sa_bo_p", [P, KC], F32), ("ca_bo_p", [P, KC], F32),
        ("ff_b2_p", [P, KC], F32),
        ("ff_b1a_p", [P, FJ], F32), ("ff_b1g_p", [P, FJ], F32),
        ("ones_f", [P, P], BF16), ("ones_fr", [P, P], F32R),
        ("e0m", [P, P], BF16),
        ("zeros_nw", [P, NW], BF16), ("epst", [P, 1], F32),
    ]
    if apply_gb:
        for ln in (1, 2, 3):
            dt_in.append((f"ln{ln}_g_p", [P, KC], F32))
            dt_in.append((f"ln{ln}_b_p", [P, KC], F32))
    nc._kd = {}
    for name, shape, dt in dt_in:
        nc._kd[name] = nc.declare_dram_parameter(name, shape, dt,
                                                 isOutput=False)
    nc._kd["outT"] = nc.declare_dram_parameter("outT", [D, NT], BF16,
                                               isOutput=True)
    with tile.TileContext(nc) as tc:
        _emit(nc, tc, apply_gb)
    nc.compile()
    return nc


def _prep_in_maps(inputs, apply_gb):
    f32 = np.float32
    bf = ml_dtypes.bfloat16
    x = np.asarray(inputs["x"], f32)
    ctx = np.asarray(inputs["context"], f32)

    def heads(w):
        # [640, 640] -> [80, 8, 640] head-major partition layout
        return np.ascontiguousarray(
            np.asarray(w, f32).reshape(NH, DH, D).transpose(1, 0, 2)
        ).astype(bf)

    def part(v, cols):
        return np.ascontiguousarray(np.asarray(v, f32).reshape(cols, P).T)

    shared = {
        "sa_wq": np.asarray(inputs["sa_wq"], f32).astype(bf),
        "sa_wk": np.asarray(inputs["sa_wk"], f32).astype(bf),
        "sa_wv": np.asarray(inputs["sa_wv"], f32).astype(bf),
        "sa_wo_h": heads(inputs["sa_wo"]),
        "ca_wq": np.asarray(inputs["ca_wq"], f32).astype(bf),
        "ca_wk": np.asarray(inputs["ca_wk"], f32).astype(bf),
        "ca_wv": np.asarray(inputs["ca_wv"], f32).astype(bf),
        "ca_wo_h": heads(inputs["ca_wo"]),
        "ff_w1": np.asarray(inputs["ff_w1"], f32).astype(bf),
        "ff_w2": np.asarray(inputs["ff_w2"], f32).astype(bf),
        "sa_bo_p": part(inputs["sa_bo"], KC),
        "ca_bo_p": part(inputs["ca_bo"], KC),
        "ff_b2_p": part(inputs["ff_b2"], KC),
        "ff_b1a_p": part(np.asarray(inputs["ff_b1"], f32)[:FH], FJ),
        "ff_b1g_p": part(np.asarray(inputs["ff_b1"], f32)[FH:], FJ),
        "ones_f": np.ones((P, P), bf),
        "ones_fr": np.ones((P, P), f32),
        "e0m": np.concatenate([np.ones((1, P), bf),
                               np.zeros((P - 1, P), bf)], axis=0),
        "zeros_nw": np.zeros((P, NW), bf),
        "epst": np.full((P, 1), LN_EPS, f32),
    }
    if apply_gb:
        for ln in (1, 2, 3):
            shared[f"ln{ln}_g_p"] = part(inputs[f"ln{ln}_g"], KC)
            shared[f"ln{ln}_b_p"] = part(inputs[f"ln{ln}_b"], KC)
    maps = []
    for i in range(B):
        m = dict(shared)
        m["xT"] = np.ascontiguousarray(x[i].T).astype(bf)
        m["ctxT_bf"] = np.ascontiguousarray(ctx[i].T).astype(bf)
        maps.append(m)
    return maps


def _needs_gb(inputs):
    for ln in (1, 2, 3):
        if not np.allclose(np.asarray(inputs[f"ln{ln}_g"]), 1.0):
            return True
        if not np.allclose(np.asarray(inputs[f"ln{ln}_b"]), 0.0):
            return True
    return False


def _run(inputs, trace=False):
    apply_gb = _needs_gb(inputs)
    nc = _build(apply_gb)
    maps = _prep_in_maps(inputs, apply_gb)
    res = run_bass_kernel_spmd(nc, maps, core_ids=list(range(B)), trace=trace)
    out = np.stack([np.asarray(r["outT"]).T for r in res.results])
    return out.astype(np.float32), res


def kernel(**inputs):
    out, _ = _run(inputs, trace=False)
    return out
